# revision 5
# baseline (speedup 1.0000x reference)
"""AttnGCN layer on 8 TRN2 NeuronCores — data-parallel over batch.

Per-core (one sample b):
  q = x @ Wq + bq ; k = x @ Wk + bk            (fp8 DoubleRow PE matmuls)
  sT[i,o] = k_i . q_o  + C'*eT[i,o]            (scores transposed; mask folded
                                                into PSUM via lhsT=e-block
                                                matmuls against a scaled fp8
                                                identity — transposes e free)
  pT = exp(alpha*sT - C)                        (ACT, masked entries -> ~0)
  S[o] = sum_i pT[i,o]                          (ones-vector PE matmul)
  ctxT[e,o] = sum_i x8[i,e] * pT[i,o]           (PE fp8 DR, accumulated over i)
  out_pre[o,c] = sum_e ctx8[e,o] * Wc8[e,c]     (PE fp8 DR) ; scale rows by 1/S
  x = x + out_pre ; LayerNorm(x)*gamma + beta   (DVE bn_stats/bn_aggr +
                                                Newton rsqrt epilogue)

Self-contained: hardcodes shapes from the problem spec.
"""

import math
from contextlib import ExitStack

import numpy as np

import concourse.bass as bass
import concourse.tile as tile
from concourse import mybir
from concourse.vector_clock import ScopedClock

F32 = mybir.dt.float32
I32 = mybir.dt.int32
BF16 = mybir.dt.bfloat16
FP8 = mybir.dt.float8e4

B = 8
N = 2048
D = 512
P = 128
NB = N // P       # 16 i-blocks
EC = D // P       # 4 chunks of the embed/dff dim
OC = N // 512     # 4 o-chunks of 512 attn rows
ALPHA = 1.0 / math.sqrt(D)
CPRIME = 128.0            # mask scale inside PSUM (fp8e4 max finite is 240)
SHIFT = 3.0               # softmax-invariant shift keeping exp() in fp8 range
CBIAS = CPRIME * ALPHA + SHIFT  # subtracted in the exp bias
RSQRT_MAGIC = 0x5F3759DF


# ---------------------------------------------------------------------------
# Workaround: walrus CoreV3 rejects >2 sem waits on the TileContext final
# drain ("Too many sync wait commands"). Hoist waits onto preceding nops.
def _patched_drain_and_barrier(self, tick_clock, wait_clock):
    nc = self.nc
    carrier = nc.sync.nop(nofuse=True)
    wait_clock.add_sem_waits(carrier.ins, ScopedClock({None: tick_clock.global_clock}))
    si = carrier.ins.sync_info
    waits = list(si.on_wait) if si and si.on_wait else []
    if len(waits) > 1:
        si.on_wait = waits[:1]
        for w in waits[1:]:
            n2 = nc.sync.nop(nofuse=True)
            n2.ins.sync_info = mybir.SyncInfo(on_wait=[w], on_update=[])
    nc.sync.drain()
    nc.all_engine_barrier()
    assert self.sems is not None
    popped = nc._tile_sem_poison_stack.pop()
    assert popped is self._sem_poison
    nc.clear_and_free_semaphores(list(self.sems.allocated().values()))
    nc.all_engine_barrier()


def _apply_patches():
    tile.TileContext._drain_and_barrier = _patched_drain_and_barrier


def _split_excess_waits(nc, limit=1):
    """walrus CoreV2/V3 codegen rejects instructions with >2 sem waits;
    hoist excess waits onto same-engine no-ops inserted just before."""
    n = 0
    for fn in nc.m.functions:
        for blk in fn.blocks:
            out = []
            changed = False
            for inst in blk.instructions:
                si = inst.sync_info
                waits = list(si.on_wait) if si and si.on_wait else []
                if len(waits) > limit:
                    keep = waits[-limit:]
                    for w in waits[:-limit]:
                        n += 1
                        nop = mybir.InstNoOp(name=f"I-wsplit-{n}", ins=[], outs=[])
                        nop.engine = inst.engine
                        nop.sync_info = mybir.SyncInfo(on_wait=[w], on_update=[])
                        out.append(nop)
                    si.on_wait = keep
                    changed = True
                out.append(inst)
            if changed:
                blk.instructions = out
    return n


def _identity(nc, ap, diag):
    nc.gpsimd.memset(ap, 0.0)
    nc.gpsimd.affine_select(
        out=ap,
        in_=ap,
        compare_op=mybir.AluOpType.not_equal,
        fill=diag,
        base=0,
        pattern=[[-1, ap.shape[0]]],
        channel_multiplier=1,
    )


def build_nc():
    nc = bass.Bass()
    x_ext = nc.declare_dram_parameter("node_fts", [N, D], F32, isOutput=False)
    e_ext = nc.declare_dram_parameter("rel_edges", [N, N], F32, isOutput=False)
    wq_ext = nc.declare_dram_parameter("Wq", [D, D], F32, isOutput=False)
    bq_ext = nc.declare_dram_parameter("bq", [D], F32, isOutput=False)
    wk_ext = nc.declare_dram_parameter("Wk", [D, D], F32, isOutput=False)
    bk_ext = nc.declare_dram_parameter("bk", [D], F32, isOutput=False)
    wc_ext = nc.declare_dram_parameter("Wc", [D, D], F32, isOutput=False)
    g_ext = nc.declare_dram_parameter("gamma", [D], F32, isOutput=False)
    be_ext = nc.declare_dram_parameter("beta", [D], F32, isOutput=False)
    out_ext = nc.declare_dram_parameter("out", [N, D], F32, isOutput=True)

    with tile.TileContext(nc) as tc, ExitStack() as ctx:
        singles = ctx.enter_context(tc.tile_pool(name="singles", bufs=1))
        wstage = ctx.enter_context(tc.tile_pool(name="wstage", bufs=2))
        xtp = ctx.enter_context(tc.tile_pool(name="xtp", bufs=2))
        efp = ctx.enter_context(tc.tile_pool(name="efp", bufs=4))
        e8p = ctx.enter_context(tc.tile_pool(name="e8p", bufs=6))
        ptp = ctx.enter_context(tc.tile_pool(name="ptp", bufs=4))
        ctxp = ctx.enter_context(tc.tile_pool(name="ctxp", bufs=2))
        rowp = ctx.enter_context(tc.tile_pool(name="rowp", bufs=2))
        epi = ctx.enter_context(tc.tile_pool(name="epi", bufs=3))
        xsbp = ctx.enter_context(tc.tile_pool(name="xsbp", bufs=5))
        sps = ctx.enter_context(tc.tile_pool(name="sps", bufs=3, space="PSUM"))
        ctxps_pool = ctx.enter_context(tc.tile_pool(name="ctxps", bufs=1, space="PSUM"))
        spsum = ctx.enter_context(tc.tile_pool(name="spsum", bufs=1, space="PSUM"))

        # ---- persistent tiles -------------------------------------------
        xs4 = [
            singles.tile([P, 4, D], F32, tag=f"xs{g}", name=f"xs{g}")
            for g in range(4)
        ]
        x8g = [
            singles.tile([P, 4, D], FP8, tag=f"x8g{g}", name=f"x8g{g}")
            for g in range(4)
        ]
        qt8 = singles.tile([P, EC, N], FP8, tag="qt8")
        kt8 = singles.tile([P, EC, N], FP8, tag="kt8")
        wq8 = singles.tile([P, EC, D], FP8, tag="wq8")
        wk8 = singles.tile([P, EC, D], FP8, tag="wk8")
        wc8 = singles.tile([P, EC, D], FP8, tag="wc8")
        bqt = singles.tile([P, EC], F32, tag="bqt")
        bkt = singles.tile([P, EC], F32, tag="bkt")
        gamma_b = singles.tile([P, D], F32, tag="gamma_b")
        beta_b = singles.tile([P, D], F32, tag="beta_b")
        ident32 = singles.tile([P, P], F32, tag="ident32")
        maskid8 = singles.tile([P, P], FP8, tag="maskid8")
        ones8 = singles.tile([P, 2, 16], FP8, tag="ones8")
        one32 = singles.tile([1, 1], F32, tag="one32")
        cbias_t = singles.tile([P, 1], F32, tag="cbias_t")

        _identity(nc, ident32, 1.0)
        nc.vector.tensor_scalar(
            out=maskid8, in0=ident32, scalar1=CPRIME, scalar2=None,
            op0=mybir.AluOpType.mult,
        )
        nc.gpsimd.memset(ones8, 1.0)
        nc.gpsimd.memset(one32, 1.0)
        nc.gpsimd.memset(cbias_t, -CBIAS)

        # ---- t=0 DMAs: x on sync ring; weights/biases on scalar ring ----
        for g in range(4):
            nc.sync.dma_start(
                out=xs4[g],
                in_=x_ext[g * 4 * P : (g + 1) * 4 * P, :].rearrange(
                    "(ib p) e -> p ib e", p=P
                ),
            )
        wq_st = wstage.tile([P, EC, D], F32, tag="wstage", name="wq_st")
        nc.scalar.dma_start(
            out=wq_st, in_=wq_ext[:, :].rearrange("(ec p) f -> p ec f", p=P)
        )
        wk_st = wstage.tile([P, EC, D], F32, tag="wstage", name="wk_st")
        nc.scalar.dma_start(
            out=wk_st, in_=wk_ext[:, :].rearrange("(ec p) f -> p ec f", p=P)
        )
        # biases laid out per-partition: b[f] -> [p, fc] with f = fc*128 + p
        nc.scalar.dma_start(out=bqt, in_=bq_ext[:].rearrange("(fc p) -> p fc", p=P))
        nc.scalar.dma_start(out=bkt, in_=bk_ext[:].rearrange("(fc p) -> p fc", p=P))
        ge = g_ext[:]
        nc.scalar.dma_start(
            out=gamma_b,
            in_=bass.AP(tensor=ge.tensor, offset=ge.offset, ap=[[0, P], *ge.ap]),
        )
        bea = be_ext[:]
        nc.scalar.dma_start(
            out=beta_b,
            in_=bass.AP(tensor=bea.tensor, offset=bea.offset, ap=[[0, P], *bea.ap]),
        )

        # edge loader: one quarter of an o-chunk's mask columns at a time.
        # f32 staging lands via alternating DMA rings; DVE casts to fp8.
        # gate=True delays the DMA descriptor push until most of x has
        # landed (WAW dep on a tiny gpsimd write) so the oc0 edge loads
        # don't steal HBM bandwidth from the prep-critical x/w loads.
        def emit_e_dma(oc, q, gate=False):
            ef = efp.tile([P, 4, 512], F32, tag="ef", name=f"ef{oc}{q}")
            if gate:
                nc.gpsimd.tensor_copy(out=ef[0:1, 0, 0:1], in_=xs4[2][0:1, 0, 0:1])
            eng = nc.sync if (oc * 4 + q) % 2 == 0 else nc.scalar
            eng.dma_start(
                out=ef,
                in_=e_ext[
                    oc * 512 : (oc + 1) * 512, q * 512 : (q + 1) * 512
                ].rearrange("(s p) f -> p s f", p=P),
            )
            return ef

        def emit_e_cast(oc, q, ef):
            e8 = e8p.tile([P, 4, 512], FP8, tag="e8", name=f"e8{oc}{q}")
            nc.vector.tensor_copy(out=e8, in_=ef)
            return e8

        # x natural-layout fp8 casts (ctx matmul lhsT); x8g[0] first so the
        # gpsimd queue reaches the edge-DMA gates right as xs4[2] lands
        nc.gpsimd.tensor_copy(out=x8g[0], in_=xs4[0])

        # oc0 edge DMAs, gated behind the x loads
        ef_pre = [emit_e_dma(0, q, gate=True) for q in range(4)]

        nc.gpsimd.tensor_copy(out=x8g[1], in_=xs4[1])
        nc.vector.tensor_copy(out=x8g[2], in_=xs4[2])
        nc.gpsimd.tensor_copy(out=x8g[3], in_=xs4[3])

        # ---- HAM warmup: dummy matmul burst while the first DMAs land ----
        # PE clock-gate needs ~3.4us of sustained activity to go 1.2->2.4GHz;
        # burn the DMA-wait with throwaway matmuls so prep runs warm.
        warm_ps = sps.tile([P, 512], F32, tag="sps")
        for j in range(104):
            nc.tensor.matmul(
                out=warm_ps[:, (j % 4) * P : (j % 4 + 1) * P],
                lhsT=maskid8,
                rhs=maskid8,
                start=True,
                stop=True,
                skip_group_check=True,
            )

        # ---- stage q/k weights -> fp8 (ACT copies) ----------------------
        nc.scalar.copy(out=wq8, in_=wq_st)
        nc.scalar.copy(out=wk8, in_=wk_st)

        # ---- stage xT + projections qT[f,i], kT[f,i] (fp8 DoubleRow) ----
        for g in range(4):
            xs = xs4[g]
            xt = xtp.tile([P, EC, 512], FP8, tag="xt")
            for ec in range(EC):
                tp = sps.tile([P, 512], F32, tag="sps")
                for k4 in range(4):
                    nc.tensor.transpose(
                        out=tp[:, k4 * P : (k4 + 1) * P],
                        in_=xs[:, k4, ec * P : (ec + 1) * P],
                        identity=ident32,
                    )
                nc.vector.tensor_copy(out=xt[:, ec, :], in_=tp)
            for w8, bt, dst, eng in (
                (wq8, bqt, qt8, "act"),
                (wk8, bkt, kt8, "dve"),
            ):
                for fc in range(EC):
                    ps = sps.tile([P, 512], F32, tag="sps")
                    for dc in (0, 2):
                        nc.tensor.matmul(
                            out=ps,
                            lhsT=w8[:, dc : dc + 2, fc * P : (fc + 1) * P],
                            rhs=xt[:, dc : dc + 2, :],
                            start=(dc == 0),
                            stop=(dc == 2),
                            perf_mode=mybir.MatmulPerfMode.DoubleRow,
                            skip_group_check=True,
                        )
                    if eng == "act":
                        nc.scalar.activation(
                            out=dst[:, fc, g * 512 : (g + 1) * 512],
                            in_=ps,
                            func=mybir.ActivationFunctionType.Identity,
                            bias=bt[:, fc : fc + 1],
                            scale=1.0,
                        )
                    else:
                        nc.vector.tensor_scalar(
                            out=dst[:, fc, g * 512 : (g + 1) * 512],
                            in0=ps,
                            scalar1=bt[:, fc : fc + 1],
                            scalar2=None,
                            op0=mybir.AluOpType.add,
                        )

        # deferred Wc staging (first used at the oc0 tail)
        wc_st = wstage.tile([P, EC, D], F32, tag="wstage", name="wc_st")
        nc.scalar.dma_start(
            out=wc_st, in_=wc_ext[:, :].rearrange("(ec p) f -> p ec f", p=P)
        )
        nc.scalar.copy(out=wc8, in_=wc_st)

        # ---- main loop over o-chunks ------------------------------------
        for oc in range(OC):
            if oc == 0:
                ef_q = ef_pre
            # prefetch next oc's edge DMAs right away
            if oc + 1 < OC:
                ef_next = [emit_e_dma(oc + 1, q) for q in range(4)]

            e8_q = [emit_e_cast(oc, q, ef_q[q]) for q in range(4)]
            if oc + 1 < OC:
                ef_q = ef_next

            ctx_ps = ctxps_pool.tile([P, EC, 512], F32, tag="ctxps")
            s_ps = spsum.tile([1, 512], F32, tag="spsum")

            pt2 = None
            for ib in range(NB):
                e8, il = e8_q[ib // 4], ib % 4
                sp = sps.tile([P, 512], F32, tag="sps")
                for s in range(4):
                    # start=True clears the whole PSUM bank -> only on s==0;
                    # later mask MMs hit has_written=0 and write directly.
                    nc.tensor.matmul(
                        out=sp[:, s * P : (s + 1) * P],
                        lhsT=e8[:, s, il * P : (il + 1) * P],
                        rhs=maskid8,
                        start=(s == 0),
                        stop=False,
                        skip_group_check=True,
                    )
                for dc in (0, 2):
                    nc.tensor.matmul(
                        out=sp,
                        lhsT=kt8[:, dc : dc + 2, ib * P : (ib + 1) * P],
                        rhs=qt8[:, dc : dc + 2, oc * 512 : (oc + 1) * 512],
                        start=False,
                        stop=(dc == 2),
                        perf_mode=mybir.MatmulPerfMode.DoubleRow,
                        skip_group_check=True,
                    )
                if ib % 2 == 0:
                    pt2 = ptp.tile([P, 2, 512], FP8, tag="pt")
                nc.scalar.activation(
                    out=pt2[:, ib % 2, :],
                    in_=sp,
                    func=mybir.ActivationFunctionType.Exp,
                    bias=cbias_t[:, 0:1],
                    scale=ALPHA,
                )
                if ib % 2 == 1:
                    j = (ib % 4) - 1
                    for ec in range(EC):
                        nc.tensor.matmul(
                            out=ctx_ps[:, ec, :],
                            lhsT=x8g[ib // 4][:, j : j + 2, ec * P : (ec + 1) * P],
                            rhs=pt2,
                            start=(ib == 1),
                            stop=(ib == NB - 1),
                            perf_mode=mybir.MatmulPerfMode.DoubleRow,
                            skip_group_check=True,
                        )
                    nc.tensor.matmul(
                        out=s_ps,
                        lhsT=ones8[:, :, 0:1],
                        rhs=pt2,
                        start=(ib == 1),
                        stop=(ib == NB - 1),
                        perf_mode=mybir.MatmulPerfMode.DoubleRow,
                        skip_group_check=True,
                    )

            # unnormalized ctx -> SBUF fp8 (split ACT/DVE to shorten the
            # PE stall before the Wc matmuls)
            ctx8 = ctxp.tile([P, EC, 512], FP8, tag="ctx8")
            for ec in range(EC):
                if ec % 2 == 0:
                    nc.scalar.copy(out=ctx8[:, ec, :], in_=ctx_ps[:, ec, :])
                else:
                    nc.vector.tensor_copy(out=ctx8[:, ec, :], in_=ctx_ps[:, ec, :])

            s_sb = rowp.tile([1, 512], F32, tag="s_sb")
            nc.vector.tensor_copy(out=s_sb, in_=s_ps)

            # out_pre = ctx_unnorm @ Wc ; scale rows by 1/S ; residual + LN
            # pass 1: x = out_pre/S + xres (one fused DVE op, sum via accum);
            # sum(x^2) on ACT Square-accum -> mean/var without bn_stats
            x_tiles = []
            msum = epi.tile([P, 4], F32, tag="msum")
            qsum = epi.tile([P, 4], F32, tag="qsum")
            s_col = None
            rs_col = None
            for os4 in range(4):
                opre = sps.tile([P, 512], F32, tag="sps")
                for dc in (0, 2):
                    nc.tensor.matmul(
                        out=opre,
                        lhsT=ctx8[:, dc : dc + 2, os4 * P : (os4 + 1) * P],
                        rhs=wc8[:, dc : dc + 2, :],
                        start=(dc == 0),
                        stop=(dc == 2),
                        perf_mode=mybir.MatmulPerfMode.DoubleRow,
                        skip_group_check=True,
                    )
                if os4 == 0:
                    # 1/S per-partition: S row -> PE transpose -> recip;
                    # emitted after the first Wc group so PE never waits
                    # on the DVE S-copy.
                    s_col = sps.tile([P, 4], F32, tag="sps")
                    for j in range(4):
                        nc.tensor.matmul(
                            out=s_col[:, j : j + 1],
                            lhsT=s_sb[0:1, j * P : (j + 1) * P],
                            rhs=one32,
                            is_transpose=True,
                            start=(j == 0),
                            stop=(j == 3),
                            skip_group_check=True,
                        )
                    rs_col = rowp.tile([P, 4], F32, tag="rs_col")
                    nc.vector.reciprocal(out=rs_col, in_=s_col)
                x_sb = xsbp.tile([P, D], F32, tag="x_sb")
                nc.vector.scalar_tensor_tensor(
                    out=x_sb,
                    in0=opre,
                    scalar=rs_col[:, os4 : os4 + 1],
                    in1=xs4[oc][:, os4, :],
                    op0=mybir.AluOpType.mult,
                    op1=mybir.AluOpType.add,
                    accum_out=msum[:, os4 : os4 + 1],
                )
                x_tiles.append(x_sb)
                sq_scr = epi.tile([P, D], F32, tag="sq_scr")
                nc.scalar.activation(
                    out=sq_scr,
                    in_=x_sb,
                    func=mybir.ActivationFunctionType.Square,
                    accum_out=qsum[:, os4 : os4 + 1],
                )

            # mean = msum/512 ; var = qsum/512 - mean^2 ; then
            # rstd = 1/sqrt(var+eps) via Newton iterations on DVE (avoids
            # ACT Sqrt -> no activation-table thrash against the Exps)
            mu4 = epi.tile([P, 4], F32, tag="mu4")
            nc.vector.tensor_scalar(
                out=mu4, in0=msum, scalar1=1.0 / D, scalar2=None,
                op0=mybir.AluOpType.mult,
            )
            q4e = epi.tile([P, 4], F32, tag="q4e")
            nc.vector.tensor_scalar(
                out=q4e, in0=qsum, scalar1=1.0 / D, scalar2=1e-5,
                op0=mybir.AluOpType.mult, op1=mybir.AluOpType.add,
            )
            v_eps = epi.tile([P, 4], F32, tag="v_eps")
            nc.vector.scalar_tensor_tensor(
                out=v_eps, in0=mu4, scalar=-1.0, in1=mu4,
                op0=mybir.AluOpType.mult, op1=mybir.AluOpType.mult,
            )
            nc.vector.tensor_add(v_eps, v_eps, q4e)
            sh = epi.tile([P, 4], I32, tag="sh")
            nc.vector.tensor_scalar(
                out=sh, in0=v_eps.bitcast(I32), scalar1=1, scalar2=None,
                op0=mybir.AluOpType.logical_shift_right,
            )
            shn = epi.tile([P, 4], I32, tag="shn")
            nc.vector.tensor_scalar(
                out=shn, in0=sh, scalar1=-1, scalar2=None,
                op0=mybir.AluOpType.bitwise_xor,
            )
            y = epi.tile([P, 4], F32, tag="y0")
            nc.vector.tensor_scalar(
                out=y.bitcast(I32), in0=shn, scalar1=RSQRT_MAGIC + 1,
                scalar2=None, op0=mybir.AluOpType.add,
            )
            for it in range(2):
                a = epi.tile([P, 4], F32, tag=f"nt_a{it}")
                nc.vector.tensor_mul(a, v_eps, y)
                bb = epi.tile([P, 4], F32, tag=f"nt_b{it}")
                nc.vector.tensor_mul(bb, a, y)
                cc = epi.tile([P, 4], F32, tag=f"nt_c{it}")
                nc.vector.tensor_scalar(
                    out=cc, in0=bb, scalar1=-0.5, scalar2=1.5,
                    op0=mybir.AluOpType.mult, op1=mybir.AluOpType.add,
                )
                y2 = epi.tile([P, 4], F32, tag=f"nt_y{it}")
                nc.vector.tensor_mul(y2, y, cc)
                y = y2
            rs4 = y

            # pass 2: normalize on ACT (scale=rstd, bias=-mu*rstd),
            # gamma (DVE), beta (gpsimd)
            b4 = epi.tile([P, 4], F32, tag="b4")
            nc.vector.scalar_tensor_tensor(
                out=b4, in0=mu4, scalar=-1.0, in1=rs4,
                op0=mybir.AluOpType.mult, op1=mybir.AluOpType.mult,
            )
            for os4 in range(4):
                t_sb = epi.tile([P, D], F32, tag="t_sb")
                nc.scalar.activation(
                    out=t_sb,
                    in_=x_tiles[os4],
                    func=mybir.ActivationFunctionType.Identity,
                    bias=b4[:, os4 : os4 + 1],
                    scale=rs4[:, os4 : os4 + 1],
                )
                g_sb = epi.tile([P, D], F32, tag="g_sb")
                nc.vector.tensor_mul(g_sb, t_sb, gamma_b)
                o_sb = epi.tile([P, D], F32, tag="o_sb")
                nc.gpsimd.tensor_add(o_sb, g_sb, beta_b)
                r0 = (oc * 4 + os4) * P
                nc.sync.dma_start(out=out_ext[r0 : r0 + P, :], in_=o_sb)

    _split_excess_waits(nc)
    return nc


_NC_CACHE = None


def kernel(**inputs) -> np.ndarray:
    global _NC_CACHE
    _apply_patches()
    from concourse.bass_utils import run_bass_kernel_spmd

    node_fts = np.ascontiguousarray(np.asarray(inputs["node_fts"], dtype=np.float32))
    rel_edges = np.ascontiguousarray(np.asarray(inputs["rel_edges"], dtype=np.float32))
    shared = {
        k: np.ascontiguousarray(np.asarray(inputs[k], dtype=np.float32))
        for k in ("Wq", "bq", "Wk", "bk", "Wc", "gamma", "beta")
    }
    if _NC_CACHE is None:
        _NC_CACHE = build_nc()
    in_maps = [
        {"node_fts": node_fts[b], "rel_edges": rel_edges[b], **shared}
        for b in range(B)
    ]
    res = run_bass_kernel_spmd(_NC_CACHE, in_maps, core_ids=list(range(B)))
    return np.stack([res.results[b]["out"] for b in range(B)]).astype(np.float32)


# revision 7
# speedup vs baseline: 1.0463x; 1.0463x over previous
"""AttnGCN layer on 8 TRN2 NeuronCores — data-parallel over batch.

Per-core (one sample b):
  q = x @ Wq + bq ; k = x @ Wk + bk            (fp8 DoubleRow PE matmuls)
  sT[i,o] = k_i . q_o  + C'*eT[i,o]            (scores transposed; mask folded
                                                into PSUM via lhsT=e-block
                                                matmuls against a scaled fp8
                                                identity — transposes e free)
  pT = exp(alpha*sT - C)                        (ACT, masked entries -> ~0)
  S[o] = sum_i pT[i,o]                          (ones-vector PE matmul)
  ctxT[e,o] = sum_i x8[i,e] * pT[i,o]           (PE fp8 DR, accumulated over i)
  out_pre[o,c] = sum_e ctx8[e,o] * Wc8[e,c]     (PE fp8 DR) ; scale rows by 1/S
  x = x + out_pre ; LayerNorm(x)*gamma + beta   (fused DVE stt + ACT Square
                                                accumulators + Newton rsqrt)

q/k are built per 512-token group so the oc0 score loop starts as soon as
group 0 is projected; later groups pipeline into the loop.

Self-contained: hardcodes shapes from the problem spec.
"""

import math
from contextlib import ExitStack

import numpy as np

import concourse.bass as bass
import concourse.tile as tile
from concourse import mybir
from concourse.vector_clock import ScopedClock

F32 = mybir.dt.float32
I32 = mybir.dt.int32
BF16 = mybir.dt.bfloat16
FP8 = mybir.dt.float8e4

B = 8
N = 2048
D = 512
P = 128
NB = N // P       # 16 i-blocks
EC = D // P       # 4 chunks of the embed/dff dim
OC = N // 512     # 4 o-chunks of 512 attn rows
ALPHA = 1.0 / math.sqrt(D)
CPRIME = 128.0            # mask scale inside PSUM (fp8e4 max finite is 240)
SHIFT = 3.0               # softmax-invariant shift keeping exp() in fp8 range
CBIAS = CPRIME * ALPHA + SHIFT  # subtracted in the exp bias
RSQRT_MAGIC = 0x5F3759DF


# ---------------------------------------------------------------------------
# Workaround: walrus CoreV3 rejects >2 sem waits on the TileContext final
# drain ("Too many sync wait commands"). Hoist waits onto preceding nops.
def _patched_drain_and_barrier(self, tick_clock, wait_clock):
    nc = self.nc
    carrier = nc.sync.nop(nofuse=True)
    wait_clock.add_sem_waits(carrier.ins, ScopedClock({None: tick_clock.global_clock}))
    si = carrier.ins.sync_info
    waits = list(si.on_wait) if si and si.on_wait else []
    if len(waits) > 1:
        si.on_wait = waits[:1]
        for w in waits[1:]:
            n2 = nc.sync.nop(nofuse=True)
            n2.ins.sync_info = mybir.SyncInfo(on_wait=[w], on_update=[])
    nc.sync.drain()
    nc.all_engine_barrier()
    assert self.sems is not None
    popped = nc._tile_sem_poison_stack.pop()
    assert popped is self._sem_poison
    nc.clear_and_free_semaphores(list(self.sems.allocated().values()))
    nc.all_engine_barrier()


def _apply_patches():
    tile.TileContext._drain_and_barrier = _patched_drain_and_barrier


def _split_excess_waits(nc, limit=1):
    """walrus CoreV2/V3 codegen rejects instructions with >2 sem waits;
    hoist excess waits onto same-engine no-ops inserted just before."""
    n = 0
    for fn in nc.m.functions:
        for blk in fn.blocks:
            out = []
            changed = False
            for inst in blk.instructions:
                si = inst.sync_info
                waits = list(si.on_wait) if si and si.on_wait else []
                if len(waits) > limit:
                    keep = waits[-limit:]
                    for w in waits[:-limit]:
                        n += 1
                        nop = mybir.InstNoOp(name=f"I-wsplit-{n}", ins=[], outs=[])
                        nop.engine = inst.engine
                        nop.sync_info = mybir.SyncInfo(on_wait=[w], on_update=[])
                        out.append(nop)
                    si.on_wait = keep
                    changed = True
                out.append(inst)
            if changed:
                blk.instructions = out
    return n


def _identity(nc, ap, diag):
    nc.gpsimd.memset(ap, 0.0)
    nc.gpsimd.affine_select(
        out=ap,
        in_=ap,
        compare_op=mybir.AluOpType.not_equal,
        fill=diag,
        base=0,
        pattern=[[-1, ap.shape[0]]],
        channel_multiplier=1,
    )


def build_nc():
    nc = bass.Bass()
    x_ext = nc.declare_dram_parameter("node_fts", [N, D], F32, isOutput=False)
    e_ext = nc.declare_dram_parameter("rel_edges", [N, N], F32, isOutput=False)
    wq_ext = nc.declare_dram_parameter("Wq", [D, D], F32, isOutput=False)
    bq_ext = nc.declare_dram_parameter("bq", [D], F32, isOutput=False)
    wk_ext = nc.declare_dram_parameter("Wk", [D, D], F32, isOutput=False)
    bk_ext = nc.declare_dram_parameter("bk", [D], F32, isOutput=False)
    wc_ext = nc.declare_dram_parameter("Wc", [D, D], F32, isOutput=False)
    g_ext = nc.declare_dram_parameter("gamma", [D], F32, isOutput=False)
    be_ext = nc.declare_dram_parameter("beta", [D], F32, isOutput=False)
    out_ext = nc.declare_dram_parameter("out", [N, D], F32, isOutput=True)

    with tile.TileContext(nc) as tc, ExitStack() as ctx:
        singles = ctx.enter_context(tc.tile_pool(name="singles", bufs=1))
        wstage = ctx.enter_context(tc.tile_pool(name="wstage", bufs=2))
        xtp = ctx.enter_context(tc.tile_pool(name="xtp", bufs=4))
        efp = ctx.enter_context(tc.tile_pool(name="efp", bufs=8))
        e8p = ctx.enter_context(tc.tile_pool(name="e8p", bufs=6))
        ptp = ctx.enter_context(tc.tile_pool(name="ptp", bufs=4))
        ctxp = ctx.enter_context(tc.tile_pool(name="ctxp", bufs=2))
        rowp = ctx.enter_context(tc.tile_pool(name="rowp", bufs=2))
        epi = ctx.enter_context(tc.tile_pool(name="epi", bufs=2))
        xsbp = ctx.enter_context(tc.tile_pool(name="xsbp", bufs=5))
        sps = ctx.enter_context(tc.tile_pool(name="sps", bufs=3, space="PSUM"))
        ctxps_pool = ctx.enter_context(tc.tile_pool(name="ctxps", bufs=1, space="PSUM"))
        spsum = ctx.enter_context(tc.tile_pool(name="spsum", bufs=1, space="PSUM"))

        # ---- persistent tiles -------------------------------------------
        xs4 = [
            singles.tile([P, 4, D], F32, tag=f"xs{g}", name=f"xs{g}")
            for g in range(4)
        ]
        x8g = [
            singles.tile([P, 4, D], FP8, tag=f"x8g{g}", name=f"x8g{g}")
            for g in range(4)
        ]
        qt8g = [
            singles.tile([P, EC, 512], FP8, tag=f"qt8g{g}", name=f"qt8g{g}")
            for g in range(4)
        ]
        kt8g = [
            singles.tile([P, EC, 512], FP8, tag=f"kt8g{g}", name=f"kt8g{g}")
            for g in range(4)
        ]
        wq8 = singles.tile([P, EC, D], FP8, tag="wq8")
        wk8 = singles.tile([P, EC, D], FP8, tag="wk8")
        wc8 = singles.tile([P, EC, D], FP8, tag="wc8")
        bqt = singles.tile([P, EC], F32, tag="bqt")
        bkt = singles.tile([P, EC], F32, tag="bkt")
        gamma_b = singles.tile([P, D], F32, tag="gamma_b")
        beta_b = singles.tile([P, D], F32, tag="beta_b")
        ident32 = singles.tile([P, P], F32, tag="ident32")
        maskid8 = singles.tile([P, P], FP8, tag="maskid8")
        ones8 = singles.tile([P, 2, 16], FP8, tag="ones8")
        one32 = singles.tile([1, 1], F32, tag="one32")
        cbias_t = singles.tile([P, 1], F32, tag="cbias_t")

        _identity(nc, ident32, 1.0)
        nc.vector.tensor_scalar(
            out=maskid8, in0=ident32, scalar1=CPRIME, scalar2=None,
            op0=mybir.AluOpType.mult,
        )
        nc.gpsimd.memset(ones8, 1.0)
        nc.gpsimd.memset(one32, 1.0)
        nc.gpsimd.memset(cbias_t, -CBIAS)

        # ---- DMA issue order --------------------------------------------
        # sync ring:   xs0, wq, xs2, e01, e03 | later: out rows, e prefetch
        # scalar ring: wk, xs1, e00, xs3, e02 | later: bias/wc, e prefetch
        # x and the oc0 edge quarters interleave so token group g's k
        # projection and the matching score blocks unblock progressively.
        def dma_x(g, eng):
            eng.dma_start(
                out=xs4[g],
                in_=x_ext[g * 4 * P : (g + 1) * 4 * P, :].rearrange(
                    "(ib p) e -> p ib e", p=P
                ),
            )

        def dma_w(w_ext, name, eng):
            st = wstage.tile([P, EC, D], F32, tag="wstage", name=name)
            eng.dma_start(
                out=st, in_=w_ext[:, :].rearrange("(ec p) f -> p ec f", p=P)
            )
            return st

        ef_tiles = {}

        def emit_e_dma(oc, q, eng):
            ef = efp.tile([P, 4, 512], F32, tag="ef", name=f"ef{oc}{q}")
            eng.dma_start(
                out=ef,
                in_=e_ext[
                    oc * 512 : (oc + 1) * 512, q * 512 : (q + 1) * 512
                ].rearrange("(s p) f -> p s f", p=P),
            )
            ef_tiles[(oc, q)] = ef

        def emit_e_cast(oc, q):
            e8 = e8p.tile([P, 4, 512], FP8, tag="e8", name=f"e8{oc}{q}")
            nc.vector.tensor_copy(out=e8, in_=ef_tiles.pop((oc, q)))
            return e8

        dma_x(0, nc.sync)
        wk_st = dma_w(wk_ext, "wk_st", nc.scalar)
        wq_st = dma_w(wq_ext, "wq_st", nc.sync)
        dma_x(1, nc.scalar)
        emit_e_dma(0, 0, nc.scalar)
        dma_x(2, nc.sync)
        emit_e_dma(0, 1, nc.sync)
        dma_x(3, nc.scalar)
        emit_e_dma(0, 2, nc.scalar)
        emit_e_dma(0, 3, nc.sync)
        # biases laid out per-partition: b[f] -> [p, fc] with f = fc*128 + p
        nc.scalar.dma_start(out=bqt, in_=bq_ext[:].rearrange("(fc p) -> p fc", p=P))
        nc.scalar.dma_start(out=bkt, in_=bk_ext[:].rearrange("(fc p) -> p fc", p=P))
        ge = g_ext[:]
        nc.scalar.dma_start(
            out=gamma_b,
            in_=bass.AP(tensor=ge.tensor, offset=ge.offset, ap=[[0, P], *ge.ap]),
        )
        bea = be_ext[:]
        nc.scalar.dma_start(
            out=beta_b,
            in_=bass.AP(tensor=bea.tensor, offset=bea.offset, ap=[[0, P], *bea.ap]),
        )

        # ---- HAM warmup: dummy matmul burst while the first DMAs land ----
        warm_ps = sps.tile([P, 512], F32, tag="sps")
        for j in range(72):
            nc.tensor.matmul(
                out=warm_ps[:, (j % 4) * P : (j % 4 + 1) * P],
                lhsT=maskid8,
                rhs=maskid8,
                start=True,
                stop=True,
                skip_group_check=True,
            )

        # ---- stage q/k weights -> fp8 (ACT copies) ----------------------
        nc.scalar.copy(out=wk8, in_=wk_st)
        nc.scalar.copy(out=wq8, in_=wq_st)

        # ---- x natural-layout fp8 casts (ctx matmul lhsT) ---------------
        nc.gpsimd.tensor_copy(out=x8g[0], in_=xs4[0])
        nc.gpsimd.tensor_copy(out=x8g[1], in_=xs4[1])
        nc.vector.tensor_copy(out=x8g[2], in_=xs4[2])
        nc.gpsimd.tensor_copy(out=x8g[3], in_=xs4[3])

        # ---- xT staging + projections (fp8 DoubleRow), k first ----------
        xt_g = []

        def emit_xt(g):
            xt = xtp.tile([P, EC, 512], FP8, tag="xt", name=f"xt{g}")
            for ec in range(EC):
                tp = sps.tile([P, 512], F32, tag="sps")
                for k4 in range(4):
                    nc.tensor.transpose(
                        out=tp[:, k4 * P : (k4 + 1) * P],
                        in_=xs4[g][:, k4, ec * P : (ec + 1) * P],
                        identity=ident32,
                    )
                nc.vector.tensor_copy(out=xt[:, ec, :], in_=tp)
            xt_g.append(xt)

        def emit_proj(g, w8, bt, dst, eng):
            for fc in range(EC):
                ps = sps.tile([P, 512], F32, tag="sps")
                for dc in (0, 2):
                    nc.tensor.matmul(
                        out=ps,
                        lhsT=w8[:, dc : dc + 2, fc * P : (fc + 1) * P],
                        rhs=xt_g[g][:, dc : dc + 2, :],
                        start=(dc == 0),
                        stop=(dc == 2),
                        perf_mode=mybir.MatmulPerfMode.DoubleRow,
                        skip_group_check=True,
                    )
                if eng == "act":
                    nc.scalar.activation(
                        out=dst[:, fc, :],
                        in_=ps,
                        func=mybir.ActivationFunctionType.Identity,
                        bias=bt[:, fc : fc + 1],
                        scale=1.0,
                    )
                else:
                    nc.vector.tensor_scalar(
                        out=dst[:, fc, :],
                        in0=ps,
                        scalar1=bt[:, fc : fc + 1],
                        scalar2=None,
                        op0=mybir.AluOpType.add,
                    )

        for g in range(4):
            emit_xt(g)
            emit_proj(g, wk8, bkt, kt8g[g], "dve")
        emit_proj(0, wq8, bqt, qt8g[0], "act")

        # deferred Wc staging (first used at the oc0 tail)
        wc_st = dma_w(wc_ext, "wc_st", nc.scalar)
        nc.scalar.copy(out=wc8, in_=wc_st)

        # oc1 edge DMAs (oc+2 prefetch happens inside the loop body)
        for q in range(4):
            emit_e_dma(1, q, nc.sync if (4 + q) % 2 == 0 else nc.scalar)

        # ---- main loop over o-chunks ------------------------------------
        for oc in range(OC):
            if oc + 2 < OC:
                for q in range(4):
                    emit_e_dma(
                        oc + 2, q,
                        nc.sync if ((oc + 2) * 4 + q) % 2 == 0 else nc.scalar,
                    )

            ctx_ps = ctxps_pool.tile([P, EC, 512], F32, tag="ctxps")
            s_ps = spsum.tile([1, 512], F32, tag="spsum")

            pt2 = None
            e8 = None
            for ib in range(NB):
                il = ib % 4
                if il == 0:
                    e8 = emit_e_cast(oc, ib // 4)
                sp = sps.tile([P, 512], F32, tag="sps")
                for s in range(4):
                    # start=True clears the whole PSUM bank -> only on s==0;
                    # later mask MMs hit has_written=0 and write directly.
                    nc.tensor.matmul(
                        out=sp[:, s * P : (s + 1) * P],
                        lhsT=e8[:, s, il * P : (il + 1) * P],
                        rhs=maskid8,
                        start=(s == 0),
                        stop=False,
                        skip_group_check=True,
                    )
                for dc in (0, 2):
                    nc.tensor.matmul(
                        out=sp,
                        lhsT=kt8g[ib // 4][:, dc : dc + 2, il * P : (il + 1) * P],
                        rhs=qt8g[oc][:, dc : dc + 2, :],
                        start=False,
                        stop=(dc == 2),
                        perf_mode=mybir.MatmulPerfMode.DoubleRow,
                        skip_group_check=True,
                    )
                if ib % 2 == 0:
                    pt2 = ptp.tile([P, 2, 512], FP8, tag="pt")
                nc.scalar.activation(
                    out=pt2[:, ib % 2, :],
                    in_=sp,
                    func=mybir.ActivationFunctionType.Exp,
                    bias=cbias_t[:, 0:1],
                    scale=ALPHA,
                )
                if ib % 2 == 1:
                    j = (ib % 4) - 1
                    for ec in range(EC):
                        nc.tensor.matmul(
                            out=ctx_ps[:, ec, :],
                            lhsT=x8g[ib // 4][:, j : j + 2, ec * P : (ec + 1) * P],
                            rhs=pt2,
                            start=(ib == 1),
                            stop=(ib == NB - 1),
                            perf_mode=mybir.MatmulPerfMode.DoubleRow,
                            skip_group_check=True,
                        )
                    nc.tensor.matmul(
                        out=s_ps,
                        lhsT=ones8[:, :, 0:1],
                        rhs=pt2,
                        start=(ib == 1),
                        stop=(ib == NB - 1),
                        perf_mode=mybir.MatmulPerfMode.DoubleRow,
                        skip_group_check=True,
                    )
                # q projections for later o-chunks fill the oc0 bubbles
                if oc == 0 and il == 3 and ib < 13:
                    emit_proj(ib // 4 + 1, wq8, bqt, qt8g[ib // 4 + 1], "act")

            # unnormalized ctx -> SBUF fp8 (split ACT/DVE to shorten the
            # PE stall before the Wc matmuls)
            ctx8 = ctxp.tile([P, EC, 512], FP8, tag="ctx8")
            for ec in range(EC):
                if ec % 2 == 0:
                    nc.scalar.copy(out=ctx8[:, ec, :], in_=ctx_ps[:, ec, :])
                else:
                    nc.vector.tensor_copy(out=ctx8[:, ec, :], in_=ctx_ps[:, ec, :])

            s_sb = rowp.tile([1, 512], F32, tag="s_sb")
            nc.vector.tensor_copy(out=s_sb, in_=s_ps)

            # out_pre = ctx_unnorm @ Wc ; scale rows by 1/S ; residual + LN
            # pass 1: x = out_pre/S + xres (fused DVE op, sum via accum);
            # sum(x^2) via ACT Square-accum -> mean/var without bn_stats
            x_tiles = []
            msum = epi.tile([P, 4], F32, tag="msum")
            qsum = epi.tile([P, 4], F32, tag="qsum")
            s_col = None
            rs_col = None
            for os4 in range(4):
                opre = sps.tile([P, 512], F32, tag="sps")
                for dc in (0, 2):
                    nc.tensor.matmul(
                        out=opre,
                        lhsT=ctx8[:, dc : dc + 2, os4 * P : (os4 + 1) * P],
                        rhs=wc8[:, dc : dc + 2, :],
                        start=(dc == 0),
                        stop=(dc == 2),
                        perf_mode=mybir.MatmulPerfMode.DoubleRow,
                        skip_group_check=True,
                    )
                if os4 == 0:
                    # 1/S per-partition: S row -> PE transpose -> recip;
                    # emitted after the first Wc group so PE never waits
                    # on the DVE S-copy.
                    s_col = sps.tile([P, 4], F32, tag="sps")
                    for j in range(4):
                        nc.tensor.matmul(
                            out=s_col[:, j : j + 1],
                            lhsT=s_sb[0:1, j * P : (j + 1) * P],
                            rhs=one32,
                            is_transpose=True,
                            start=(j == 0),
                            stop=(j == 3),
                            skip_group_check=True,
                        )
                    rs_col = rowp.tile([P, 4], F32, tag="rs_col")
                    nc.vector.reciprocal(out=rs_col, in_=s_col)
                x_sb = xsbp.tile([P, D], F32, tag="x_sb")
                nc.vector.scalar_tensor_tensor(
                    out=x_sb,
                    in0=opre,
                    scalar=rs_col[:, os4 : os4 + 1],
                    in1=xs4[oc][:, os4, :],
                    op0=mybir.AluOpType.mult,
                    op1=mybir.AluOpType.add,
                    accum_out=msum[:, os4 : os4 + 1],
                )
                x_tiles.append(x_sb)
                sq_scr = epi.tile([P, D], F32, tag="sq_scr")
                nc.scalar.activation(
                    out=sq_scr,
                    in_=x_sb,
                    func=mybir.ActivationFunctionType.Square,
                    accum_out=qsum[:, os4 : os4 + 1],
                )

            # mean = msum/512 ; var = qsum/512 - mean^2 ; rstd via Newton
            # steps on DVE (no ACT Sqrt -> no table thrash)
            mu4 = epi.tile([P, 4], F32, tag="mu4")
            nc.vector.tensor_scalar(
                out=mu4, in0=msum, scalar1=1.0 / D, scalar2=None,
                op0=mybir.AluOpType.mult,
            )
            q4e = epi.tile([P, 4], F32, tag="q4e")
            nc.vector.tensor_scalar(
                out=q4e, in0=qsum, scalar1=1.0 / D, scalar2=1e-5,
                op0=mybir.AluOpType.mult, op1=mybir.AluOpType.add,
            )
            v_eps = epi.tile([P, 4], F32, tag="v_eps")
            nc.vector.scalar_tensor_tensor(
                out=v_eps, in0=mu4, scalar=-1.0, in1=mu4,
                op0=mybir.AluOpType.mult, op1=mybir.AluOpType.mult,
            )
            nc.vector.tensor_add(v_eps, v_eps, q4e)
            sh = epi.tile([P, 4], I32, tag="sh")
            nc.vector.tensor_scalar(
                out=sh, in0=v_eps.bitcast(I32), scalar1=1, scalar2=None,
                op0=mybir.AluOpType.logical_shift_right,
            )
            y = epi.tile([P, 4], F32, tag="y0")
            nc.vector.tensor_scalar(
                out=y.bitcast(I32), in0=sh, scalar1=-1, scalar2=RSQRT_MAGIC,
                op0=mybir.AluOpType.mult, op1=mybir.AluOpType.add,
            )
            for it in range(2):
                a = epi.tile([P, 4], F32, tag=f"nt_a{it}")
                nc.vector.tensor_mul(a, v_eps, y)
                bb = epi.tile([P, 4], F32, tag=f"nt_b{it}")
                nc.vector.tensor_mul(bb, a, y)
                cc = epi.tile([P, 4], F32, tag=f"nt_c{it}")
                nc.vector.tensor_scalar(
                    out=cc, in0=bb, scalar1=-0.5, scalar2=1.5,
                    op0=mybir.AluOpType.mult, op1=mybir.AluOpType.add,
                )
                y2 = epi.tile([P, 4], F32, tag=f"nt_y{it}")
                nc.vector.tensor_mul(y2, y, cc)
                y = y2
            rs4 = y

            # pass 2: normalize on ACT (scale=rstd, bias=-mu*rstd),
            # gamma (DVE), beta (gpsimd)
            b4 = epi.tile([P, 4], F32, tag="b4")
            nc.vector.scalar_tensor_tensor(
                out=b4, in0=mu4, scalar=-1.0, in1=rs4,
                op0=mybir.AluOpType.mult, op1=mybir.AluOpType.mult,
            )
            for os4 in range(4):
                t_sb = epi.tile([P, D], F32, tag="t_sb")
                nc.scalar.activation(
                    out=t_sb,
                    in_=x_tiles[os4],
                    func=mybir.ActivationFunctionType.Identity,
                    bias=b4[:, os4 : os4 + 1],
                    scale=rs4[:, os4 : os4 + 1],
                )
                g_sb = epi.tile([P, D], F32, tag="g_sb")
                nc.vector.tensor_mul(g_sb, t_sb, gamma_b)
                o_sb = epi.tile([P, D], F32, tag="o_sb")
                nc.gpsimd.tensor_add(o_sb, g_sb, beta_b)
                r0 = (oc * 4 + os4) * P
                nc.sync.dma_start(out=out_ext[r0 : r0 + P, :], in_=o_sb)

    _split_excess_waits(nc)
    return nc


_NC_CACHE = None


def kernel(**inputs) -> np.ndarray:
    global _NC_CACHE
    _apply_patches()
    from concourse.bass_utils import run_bass_kernel_spmd

    node_fts = np.ascontiguousarray(np.asarray(inputs["node_fts"], dtype=np.float32))
    rel_edges = np.ascontiguousarray(np.asarray(inputs["rel_edges"], dtype=np.float32))
    shared = {
        k: np.ascontiguousarray(np.asarray(inputs[k], dtype=np.float32))
        for k in ("Wq", "bq", "Wk", "bk", "Wc", "gamma", "beta")
    }
    if _NC_CACHE is None:
        _NC_CACHE = build_nc()
    in_maps = [
        {"node_fts": node_fts[b], "rel_edges": rel_edges[b], **shared}
        for b in range(B)
    ]
    res = run_bass_kernel_spmd(_NC_CACHE, in_maps, core_ids=list(range(B)))
    return np.stack([res.results[b]["out"] for b in range(B)]).astype(np.float32)
